# revision 1
# baseline (speedup 1.0000x reference)
"""CausalLocalAttention Trainium2 kernel (8-core SPMD, data-parallel).

Problem: B=4, S=4096, D=1024, H=16 heads, DH=64, window W=256 (block-local
causal attention), fp32 in/out.

Sharding: the 16384 tokens split into 8 contiguous 2048-token shards (block-
and batch-aligned), one per NeuronCore. Weights replicated. No collectives.

Per-core layout strategy (all matmuls fp32r, contraction on partitions):
  xT [din, t] (host-transposed)  -> QT/KT = Wq/Wk as lhsT (feature-major)
                                 -> V    = xT as lhsT    (token-major)
  S^T[k,q]  = KT-half.T @ QT  per (head, block); causal mask added on PSUM
  P^T       = exp(S^T * scale)     (ACT, fp32r)
  attn^T,l  = [V|1].T @ P^T        (M=65: row 64 = softmax denominator l)
  attn_norm = attn^T * bcast(1/l)  (PE K=1 bcast + DVE recip/mul)
  y         = attn^T as lhsT @ Wo  (token-major) -> DMA out
"""
import sys
sys.path.insert(0, "/opt/trn_rl_repo")
import os
import numpy as np
from contextlib import ExitStack

import concourse.bass as bass
import concourse.tile as tile
from concourse import mybir
from concourse.bass_utils import run_bass_kernel_spmd
from concourse.vector_clock import ScopedClock

F32 = mybir.dt.float32
F32R = mybir.dt.float32r

N_CORES = 8
B, S, D = 4, 4096, 1024
H, W, DH = 16, 256, 64
T_CORE = (B * S) // N_CORES      # 2048 tokens per core
CHUNK = 1024                     # tokens processed per chunk
N_CHUNK = T_CORE // CHUNK        # 2
SCALE = 1.0 / float(np.sqrt(DH))
NEG = -1e30


# ---------------------------------------------------------------------------
# walrus on this toolchain allows only one sem wait per instruction; split
# extras onto same-engine NoOps inserted right before the instruction.
def _split_sync_waits(nc: bass.Bass, max_waits: int = 1) -> None:
    for fn in nc.m.functions:
        for bb in fn.blocks:
            insts = bb.instructions
            if not any(
                i.sync_info and i.sync_info.on_wait
                and len(i.sync_info.on_wait) > max_waits
                for i in insts
            ):
                continue
            new = []
            for inst in insts:
                si = inst.sync_info
                waits = list(si.on_wait) if (si and si.on_wait) else []
                if len(waits) > max_waits:
                    n_excess = len(waits) - max_waits
                    for w in waits[:n_excess]:
                        nop = mybir.InstNoOp(
                            name=f"WSPLIT-{nc.next_id()}", ins=[], outs=[]
                        )
                        nop.engine = inst.engine
                        nop.sync_info = mybir.SyncInfo(on_wait=[w], on_update=[])
                        nc.register_instruction(nop)
                        new.append(nop)
                    si.on_wait = waits[n_excess:]
                new.append(inst)
            bb.instructions = new


class _WTileContext(tile.TileContext):
    def _drain_and_barrier(self, tick_clock, wait_clock):
        drain_inst = self.nc.sync.drain()
        wait_clock.add_sem_waits(
            drain_inst.ins, ScopedClock({None: tick_clock.global_clock})
        )
        self.nc.all_engine_barrier()
        assert self.sems is not None
        popped = self.nc._tile_sem_poison_stack.pop()
        assert popped is self._sem_poison
        self.nc.clear_and_free_semaphores(list(self.sems.allocated().values()))
        self.nc.all_engine_barrier()

    def __exit__(self, exc_type, exc_val, exc_tb):
        ret = super().__exit__(exc_type, exc_val, exc_tb)
        if exc_type is None:
            _split_sync_waits(self.nc)
        return ret


# ---------------------------------------------------------------------------
def build_program(repeat: int = 1, use_loop: bool = False,
                  phases=("qk", "v", "attn", "o")) -> bass.Bass:
    """Build the SPMD one-core program (same for all cores)."""
    nc = bass.Bass("TRN2", target_bir_lowering=False, debug=False,
                   num_devices=N_CORES)

    KT_N = CHUNK // 512          # moving-dim chunks per projection (2)
    NT = CHUNK // 128            # t-tiles per chunk (8)
    NB = CHUNK // W              # blocks per chunk (4)

    xt_ap = nc.dram_tensor("xt", [N_CHUNK, 128, 8, CHUNK], F32R,
                           kind="ExternalInput").ap()
    wq_ap = nc.dram_tensor("wq", [128, 8, D], F32R, kind="ExternalInput").ap()
    wk_ap = nc.dram_tensor("wk", [128, 8, D], F32R, kind="ExternalInput").ap()
    wv_ap = nc.dram_tensor("wv", [128, 8, D], F32R, kind="ExternalInput").ap()
    wo_ap = nc.dram_tensor("wo", [128, 8, D], F32R, kind="ExternalInput").ap()
    bq_ap = nc.dram_tensor("bqr", [128, 8], F32, kind="ExternalInput").ap()
    bk_ap = nc.dram_tensor("bkr", [128, 8], F32, kind="ExternalInput").ap()
    bv_ap = nc.dram_tensor("bvb", [128, D], F32, kind="ExternalInput").ap()
    bo_ap = nc.dram_tensor("bob", [128, D], F32, kind="ExternalInput").ap()
    m1_ap = nc.dram_tensor("m1", [128, W], F32, kind="ExternalInput").ap()
    m2_ap = nc.dram_tensor("m2", [128, W], F32, kind="ExternalInput").ap()
    onesb_ap = nc.dram_tensor("onesb", [1, 64], F32R, kind="ExternalInput").ap()
    y_ap = nc.dram_tensor("y", [T_CORE, D], F32, kind="ExternalOutput").ap()

    with _WTileContext(nc) as tc, ExitStack() as top:
        dma_eng = [nc.sync, nc.sync]
        consts = top.enter_context(tc.tile_pool(name="consts", bufs=1))
        m1_sb = consts.tile([128, W], F32)
        m2_sb = consts.tile([128, W], F32)
        bq_sb = consts.tile([128, 8], F32)
        bk_sb = consts.tile([128, 8], F32)
        bv_sb = consts.tile([128, D], F32)
        bo_sb = consts.tile([128, D], F32)
        onesb_sb = consts.tile([1, 64], F32R)
        nc.sync.dma_start(m1_sb[:], m1_ap[:])
        nc.sync.dma_start(m2_sb[:], m2_ap[:])
        nc.sync.dma_start(bq_sb[:], bq_ap[:])
        nc.sync.dma_start(bk_sb[:], bk_ap[:])
        nc.sync.dma_start(bv_sb[:], bv_ap[:])
        nc.sync.dma_start(bo_sb[:], bo_ap[:])
        nc.sync.dma_start(onesb_sb[:], onesb_ap[:])
        bv_h = bv_sb[:].rearrange("p (h d) -> p h d", h=H)

        rep_iter = [None] if use_loop else list(range(repeat))
        loop_cm = tc.For_i(
            0, repeat, 1,
            hint_engines=(mybir.EngineType.PE, mybir.EngineType.DVE,
                          mybir.EngineType.Activation, mybir.EngineType.SP,
                          mybir.EngineType.Pool),
        ) if use_loop else None
        if loop_cm is not None:
            loop_cm.__enter__()
        for _rep in rep_iter:
            for c in range(N_CHUNK):
                with ExitStack() as chunk_ctx:
                    p_qkv = chunk_ctx.enter_context(
                        tc.tile_pool(name="p_qkv", bufs=1))
                    p_wb = chunk_ctx.enter_context(
                        tc.tile_pool(name="p_wb", bufs=5))
                    p_x = chunk_ctx.enter_context(
                        tc.tile_pool(name="p_x", bufs=1))
                    p_work = chunk_ctx.enter_context(
                        tc.tile_pool(name="p_work", bufs=6))
                    p_y = chunk_ctx.enter_context(
                        tc.tile_pool(name="p_y", bufs=2))
                    ps = chunk_ctx.enter_context(
                        tc.tile_pool(name="ps", bufs=4, space="PSUM"))
                    ps_pv = chunk_ctx.enter_context(
                        tc.tile_pool(name="ps_pv", bufs=2, space="PSUM"))
                    ps_lb = chunk_ctx.enter_context(
                        tc.tile_pool(name="ps_lb", bufs=2, space="PSUM"))

                    qt_sb = p_qkv.tile([128, 8, CHUNK], F32R, tag="qt")
                    kt_sb = p_qkv.tile([128, 8, CHUNK], F32R, tag="kt")
                    v3_sb = p_qkv.tile([128, NT, H, DH + 1], F32R, tag="v3")

                    # xt and at share one buffer slot (disjoint lifetimes)
                    xt_sb = p_x.tile([128, 8, CHUNK], F32R, tag="xa")
                    for kk in range(2):
                        dma_eng[kk].dma_start(
                            xt_sb[:, 4 * kk:4 * (kk + 1), :],
                            xt_ap[c, :, 4 * kk:4 * (kk + 1), :])

                    def load_w(w_ap):
                        bufs = []
                        for kb in range(4):
                            wb = p_wb.tile([128, 2, D], F32R, tag="wb")
                            dma_eng[kb % 2].dma_start(
                                wb[:], w_ap[:, 2 * kb:2 * kb + 2, :])
                            bufs.append(wb)
                        return bufs

                    # ---- Q / K projections (feature-major) ----
                    if "qk" not in phases:
                        nc.vector.memset(qt_sb[:].bitcast(F32), 0.5)
                        nc.vector.memset(kt_sb[:].bitcast(F32), 0.5)
                    for w_ap, bias_sb, dst in (
                        ((wq_ap, bq_sb, qt_sb),
                         (wk_ap, bk_sb, kt_sb)) if "qk" in phases else ()):
                        wb = load_w(w_ap)
                        for m in range(8):
                            for n in range(KT_N):
                                acc = ps.tile([128, 512], F32, tag="acc")
                                for k in range(8):
                                    nc.tensor.matmul(
                                        acc[:],
                                        wb[k // 2][:, k % 2,
                                                   m * 128:(m + 1) * 128],
                                        xt_sb[:, k, n * 512:(n + 1) * 512],
                                        start=(k == 0), stop=(k == 7),
                                    )
                                nc.scalar.activation(
                                    dst[:, m, n * 512:(n + 1) * 512],
                                    acc[:],
                                    mybir.ActivationFunctionType.Identity,
                                    bias=bias_sb[:, m:m + 1],
                                )

                    # ---- V projection (token-major, with ones column) ----
                    wb = load_w(wv_ap) if "v" in phases else None
                    if "v" not in phases:
                        nc.vector.memset(v3_sb[:].bitcast(F32), 0.01)
                    for i in range(NT if "v" in phases else 0):
                        for h2 in range(2):
                            acc = ps.tile([128, 512], F32, tag="acc")
                            for k in range(8):
                                nc.tensor.matmul(
                                    acc[:],
                                    xt_sb[:, k, i * 128:(i + 1) * 128],
                                    wb[k // 2][:, k % 2,
                                               h2 * 512:(h2 + 1) * 512],
                                    start=(k == 0), stop=(k == 7),
                                )
                            nc.vector.tensor_add(
                                v3_sb[:, i, h2 * 8:(h2 + 1) * 8, 0:DH],
                                acc[:].rearrange("p (h d) -> p h d", h=8),
                                bv_h[:, h2 * 8:(h2 + 1) * 8, :],
                            )
                        nc.gpsimd.memset(
                            v3_sb[:, i, :, DH:DH + 1].bitcast(F32), 1.0)

                    # ---- attention ----
                    at_sb = p_x.tile([128, 8, CHUNK], F32R, tag="xa")
                    if "attn" not in phases:
                        nc.vector.memset(at_sb[:, 0, 0:4].bitcast(F32), 0.0)
                    for b in range(NB if "attn" in phases else 0):
                        t0 = b * W
                        for h in range(H):
                            hp = (h % 2) * 64
                            j = h // 2
                            qh = qt_sb[hp:hp + 64, j, t0:t0 + W]
                            kh = kt_sb[hp:hp + 64, j, t0:t0 + W]
                            # S^T: [k 0:128] x q 0:256 -> st[:, 0:256];
                            #      [k 128:256] x q 128:256 -> st[:, 256:384]
                            st = ps.tile([128, 2 * W], F32, tag="acc")
                            nc.tensor.matmul(
                                st[:, 0:W], kh[:, 0:128], qh[:],
                                start=True, stop=True, tile_position=(hp, 0))
                            nc.tensor.matmul(
                                st[:, W:W + 128], kh[:, 128:256],
                                qh[:, 128:256],
                                start=True, stop=True, tile_position=(hp, 0))
                            # causal masks: same triangle for both k-halves
                            nc.vector.tensor_add(st[:, 0:128], st[:, 0:128],
                                                 m1_sb[:, 0:128])
                            nc.vector.tensor_add(st[:, W:W + 128],
                                                 st[:, W:W + 128],
                                                 m1_sb[:, 0:128])
                            pt = p_work.tile([128, W + 128], F32R, tag="pt")
                            nc.scalar.activation(
                                pt[:], st[:, 0:W + 128],
                                mybir.ActivationFunctionType.Exp,
                                scale=SCALE)
                            pv = ps_pv.tile([DH + 1, W], F32, tag="pv")
                            nc.tensor.matmul(
                                pv[:], v3_sb[:, 2 * b, h, :], pt[:, 0:W],
                                start=True, stop=False)
                            nc.tensor.matmul(
                                pv[:, 128:W], v3_sb[:, 2 * b + 1, h, :],
                                pt[:, W:W + 128],
                                start=False, stop=True)
                            l_sb = p_work.tile([1, W], F32R, tag="l")
                            nc.scalar.copy(l_sb[:], pv[DH:DH + 1, :])
                            lb = ps_lb.tile([64, W], F32, tag="lb")
                            nc.tensor.matmul(lb[:], onesb_sb[:], l_sb[:],
                                             start=True, stop=True)
                            rec = p_work.tile([64, W], F32, tag="rec")
                            nc.vector.reciprocal(rec[:], lb[:])
                            nc.vector.tensor_mul(
                                at_sb[hp:hp + 64, j, t0:t0 + W],
                                pv[0:DH, :], rec[:])

                    # ---- output projection (full-row y tiles) ----
                    if "o" not in phases:
                        continue
                    wb = load_w(wo_ap)
                    for i in range(NT):
                        y_t = p_y.tile([128, D], F32, tag="y")
                        for h2 in range(2):
                            acc = ps.tile([128, 512], F32, tag="acc")
                            for k in range(8):
                                nc.tensor.matmul(
                                    acc[:],
                                    at_sb[:, k, i * 128:(i + 1) * 128],
                                    wb[k // 2][:, k % 2,
                                               h2 * 512:(h2 + 1) * 512],
                                    start=(k == 0), stop=(k == 7),
                                )
                            nc.vector.tensor_add(
                                y_t[:, h2 * 512:(h2 + 1) * 512], acc[:],
                                bo_sb[:, h2 * 512:(h2 + 1) * 512])
                        dma_eng[i % 2].dma_start(
                            y_ap[c * CHUNK + i * 128:c * CHUNK + (i + 1) * 128, :],
                            y_t[:])
        if loop_cm is not None:
            loop_cm.__exit__(None, None, None)
    return nc


# ---------------------------------------------------------------------------
_CACHE: dict = {}


def _host_prep(x, Wq, bq, Wk, bk, Wv, bv, Wo, bo):
    x = np.asarray(x, np.float32)
    Wq, Wk, Wv, Wo = (np.asarray(w, np.float32) for w in (Wq, Wk, Wv, Wo))
    bq, bk, bv, bo = (np.asarray(b, np.float32) for b in (bq, bk, bv, bo))
    x_flat = np.ascontiguousarray(x.reshape(B * S, D))
    m1 = np.zeros((128, W), np.float32)
    m2 = np.zeros((128, W), np.float32)
    for p in range(128):
        m1[p, :p] = NEG
        m2[p, :128 + p] = NEG
    def wfmt(Wm):
        # [128, 8, D]: wfmt[p, k, c] = W[k*128 + p, c]
        return np.ascontiguousarray(
            np.asarray(Wm, np.float32).reshape(8, 128, D).transpose(1, 0, 2))

    def xfmt(shard_x):
        # shard_x [T_CORE, D] -> [N_CHUNK, 128, 8, CHUNK]
        xt = shard_x.T  # [D, T_CORE]
        return np.ascontiguousarray(
            xt.reshape(8, 128, N_CHUNK, CHUNK).transpose(2, 1, 0, 3))

    shard = {
        "xt": np.stack([
            xfmt(x_flat[cix * T_CORE:(cix + 1) * T_CORE])
            for cix in range(N_CORES)
        ]),
    }
    repl = {
        "wq": wfmt(Wq),
        "wk": wfmt(Wk),
        "wv": wfmt(Wv),
        "wo": wfmt(Wo),
        "bqr": np.ascontiguousarray(np.asarray(bq, np.float32).reshape(8, 128).T),
        "bkr": np.ascontiguousarray(np.asarray(bk, np.float32).reshape(8, 128).T),
        "bvb": np.ascontiguousarray(np.tile(np.asarray(bv, np.float32), (128, 1))),
        "bob": np.ascontiguousarray(np.tile(np.asarray(bo, np.float32), (128, 1))),
        "m1": m1,
        "m2": m2,
        "onesb": np.ones((1, 64), np.float32),
    }
    return shard, repl


def _make_runner(repeat: int, use_loop: bool = False,
                 phases=("qk", "v", "attn", "o")):
    """Build program + cached jitted executable. Returns (run, n_outs info)."""
    import jax
    from jax.sharding import Mesh, PartitionSpec
    from jax.experimental.shard_map import shard_map
    from concourse import bass2jax
    from concourse.bass2jax import _bass_exec_p, install_neuronx_cc_hook

    install_neuronx_cc_hook()
    nc = build_program(repeat, use_loop, phases)
    partition_name = (
        nc.partition_id_tensor.name if nc.partition_id_tensor else None
    )
    in_names, out_names, out_avals = [], [], []
    import jax.core
    for alloc in nc.m.functions[0].allocations:
        if not isinstance(alloc, mybir.MemoryLocationSet):
            continue
        name = alloc.memorylocations[0].name
        if alloc.kind == "ExternalInput":
            if name != partition_name:
                in_names.append(name)
        elif alloc.kind == "ExternalOutput":
            out_names.append(name)
            out_avals.append(jax.core.ShapedArray(
                tuple(alloc.tensor_shape), mybir.dt.np(alloc.dtype)))
    all_in_names = list(in_names) + list(out_names)
    if partition_name is not None:
        all_in_names.append(partition_name)

    def _body(*args):
        operands = list(args)
        if partition_name is not None:
            operands.append(bass2jax.partition_id_tensor())
        return tuple(_bass_exec_p.bind(
            *operands,
            out_avals=tuple(out_avals),
            in_names=tuple(all_in_names),
            out_names=tuple(out_names),
            lowering_input_output_aliases=(),
            sim_require_finite=True,
            sim_require_nnan=True,
            nc=nc,
        ))

    import jax as _jax
    devices = _jax.devices()[:N_CORES]
    mesh = Mesh(np.asarray(devices), ("core",))
    SHARDED_INS = {"xt"}
    in_specs = tuple(
        PartitionSpec("core") if n in SHARDED_INS else PartitionSpec()
        for n in in_names
    ) + (PartitionSpec("core"),) * len(out_names)
    out_specs = (PartitionSpec("core"),) * len(out_names)
    sharded = _jax.jit(
        shard_map(_body, mesh=mesh, in_specs=in_specs,
                  out_specs=out_specs, check_rep=False),
        keep_unused=True,
    )

    from jax.sharding import NamedSharding
    sh_core = NamedSharding(mesh, PartitionSpec("core"))
    sh_repl = NamedSharding(mesh, PartitionSpec())

    def _args(shard_arrs: dict, repl_arrs: dict):
        args, shs = [], []
        for n in in_names:
            if n in SHARDED_INS:
                a = shard_arrs[n]
                args.append(a.reshape(a.shape[0] * a.shape[1], *a.shape[2:]))
                shs.append(sh_core)
            else:
                args.append(repl_arrs[n])
                shs.append(sh_repl)
        for av in out_avals:
            args.append(np.zeros((N_CORES * av.shape[0], *av.shape[1:]),
                                 av.dtype))
            shs.append(sh_core)
        return args, shs

    class Runner:
        def stage(self, shard_arrs, repl_arrs):
            args, shs = _args(shard_arrs, repl_arrs)
            dargs = [_jax.device_put(a, s) for a, s in zip(args, shs)]
            _jax.block_until_ready(dargs)
            return dargs

        def exec_staged(self, dargs):
            outs = sharded(*dargs)
            _jax.block_until_ready(outs)
            return outs

        def run(self, shard_arrs, repl_arrs):
            args, _ = _args(shard_arrs, repl_arrs)
            outs = sharded(*args)
            _jax.block_until_ready(outs)
            return {
                name: np.asarray(outs[i]).reshape(N_CORES, *out_avals[i].shape)
                for i, name in enumerate(out_names)
            }

    return Runner()


def get_runner(repeat: int = 1, use_loop: bool = False,
               phases=("qk", "v", "attn", "o")):
    key = ("runner", repeat, use_loop, tuple(phases))
    if key not in _CACHE:
        _CACHE[key] = _make_runner(repeat, use_loop, phases)
    return _CACHE[key]


def kernel(**inputs) -> np.ndarray:
    runner = get_runner(repeat=1)
    shard, repl = _host_prep(**inputs)
    out = runner.run(shard, repl)
    y = out["y"].reshape(B * S, D)
    return y.reshape(B, S, D).astype(np.float32)



# revision 4
# speedup vs baseline: 2.1922x; 2.1922x over previous
"""CausalLocalAttention Trainium2 kernel v2 (8-core SPMD, data-parallel).

Problem: B=4, S=4096, D=1024, H=16 heads, DH=64, window W=256 (block-local
causal attention), fp32 in/out.

Sharding: 16384 tokens -> 8 contiguous 2048-token shards (block-aligned),
one per NeuronCore. Weights replicated. No collectives.

v2 design (vs v1 baseline):
  - all matmul operands bf16 (fp32 PSUM accumulate): no fp32r small-matmul
    penalty, half the DMA/SBUF footprint. Tolerance is 2e-2; bf16 keeps
    max rel err ~1e-3.
  - every matmul is plain 128x128 mode (no tile_position): per-head K
    stationaries are zero-padded to 128 contraction rows so the unused
    head-half of the moving Q panel is killed by zeros, avoiding PE
    tiling-mode switches between score and PV matmuls.
  - causal mask folded into PSUM via one identity-stationary matmul that
    pre-loads the -1e30 triangle pattern (PE is cheaper per element than
    DVE here and keeps the chain on one engine).
  - softmax denominator l = colsum(P) via ones-stationary matmul into the
    same PSUM bank as PV (cols 256:512): no ACT row-copy, no broadcast op.
  - weights loaded once per core (not per chunk).
Per-core phases per 512-token chunk: K/Q/V projections (feature-major K/Q,
token-major V), per-(block,head) attention, token-major O projection.
"""
import sys
sys.path.insert(0, "/opt/trn_rl_repo")
import os
import numpy as np
from contextlib import ExitStack

import concourse.bass as bass
import concourse.tile as tile
from concourse import mybir
from concourse.bass_utils import run_bass_kernel_spmd
from concourse.vector_clock import ScopedClock

F32 = mybir.dt.float32
BF16 = mybir.dt.bfloat16
NPBF16 = mybir.dt.np(BF16)

N_CORES = 8
B, S, D = 4, 4096, 1024
H, W, DH = 16, 256, 64
T_CORE = (B * S) // N_CORES      # 2048 tokens per core
CHUNK = 512                      # tokens processed per chunk
N_CHUNK = T_CORE // CHUNK        # 4
NT = CHUNK // 128                # token tiles per chunk (4)
NB = CHUNK // W                  # attention blocks per chunk (2)
SCALE = 1.0 / float(np.sqrt(DH))
NEG = -1e30


# ---------------------------------------------------------------------------
# walrus on this toolchain allows only one sem wait per instruction; split
# extras onto same-engine NoOps inserted right before the instruction.
def _split_sync_waits(nc: bass.Bass, max_waits: int = 1) -> None:
    for fn in nc.m.functions:
        for bb in fn.blocks:
            insts = bb.instructions
            if not any(
                i.sync_info and i.sync_info.on_wait
                and len(i.sync_info.on_wait) > max_waits
                for i in insts
            ):
                continue
            new = []
            for inst in insts:
                si = inst.sync_info
                waits = list(si.on_wait) if (si and si.on_wait) else []
                if len(waits) > max_waits:
                    n_excess = len(waits) - max_waits
                    for w in waits[:n_excess]:
                        nop = mybir.InstNoOp(
                            name=f"WSPLIT-{nc.next_id()}", ins=[], outs=[]
                        )
                        nop.engine = inst.engine
                        nop.sync_info = mybir.SyncInfo(on_wait=[w], on_update=[])
                        nc.register_instruction(nop)
                        new.append(nop)
                    si.on_wait = waits[n_excess:]
                new.append(inst)
            bb.instructions = new


class _WTileContext(tile.TileContext):
    def _drain_and_barrier(self, tick_clock, wait_clock):
        drain_inst = self.nc.sync.drain()
        wait_clock.add_sem_waits(
            drain_inst.ins, ScopedClock({None: tick_clock.global_clock})
        )
        self.nc.all_engine_barrier()
        assert self.sems is not None
        popped = self.nc._tile_sem_poison_stack.pop()
        assert popped is self._sem_poison
        self.nc.clear_and_free_semaphores(list(self.sems.allocated().values()))
        self.nc.all_engine_barrier()

    def __exit__(self, exc_type, exc_val, exc_tb):
        ret = super().__exit__(exc_type, exc_val, exc_tb)
        if exc_type is None:
            _split_sync_waits(self.nc)
        return ret


# ---------------------------------------------------------------------------
def build_program(repeat: int = 1, use_loop: bool = False,
                  phases=("qk", "v", "attn", "o")) -> bass.Bass:
    """Build the SPMD one-core program (same for all cores)."""
    nc = bass.Bass("TRN2", target_bir_lowering=False, debug=False,
                   num_devices=N_CORES)

    xt_ap = nc.dram_tensor("xt", [N_CHUNK, 128, 8, CHUNK], BF16,
                           kind="ExternalInput").ap()
    wq_ap = nc.dram_tensor("wq", [128, 8, D], BF16, kind="ExternalInput").ap()
    wk_ap = nc.dram_tensor("wk", [128, 8, D], BF16, kind="ExternalInput").ap()
    wv_ap = nc.dram_tensor("wv", [128, 8, D], BF16, kind="ExternalInput").ap()
    wo_ap = nc.dram_tensor("wo", [128, 8, D], BF16, kind="ExternalInput").ap()
    bq_ap = nc.dram_tensor("bqr", [128, 8], F32, kind="ExternalInput").ap()
    bk_ap = nc.dram_tensor("bkr", [128, 8], F32, kind="ExternalInput").ap()
    bv_ap = nc.dram_tensor("bvh", [128, H, DH], BF16, kind="ExternalInput").ap()
    bo_ap = nc.dram_tensor("bob", [128, D], BF16, kind="ExternalInput").ap()
    m3_ap = nc.dram_tensor("m3", [128, 384], BF16, kind="ExternalInput").ap()
    i128_ap = nc.dram_tensor("i128", [128, 128], BF16,
                             kind="ExternalInput").ap()
    ones_ap = nc.dram_tensor("ones64", [128, 64], BF16,
                             kind="ExternalInput").ap()
    y_ap = nc.dram_tensor("y", [T_CORE, D], F32, kind="ExternalOutput").ap()

    with _WTileContext(nc) as tc, ExitStack() as top:
        consts = top.enter_context(tc.tile_pool(name="consts", bufs=1))
        wq_sb = consts.tile([128, 8, D], BF16, tag="wq")
        wk_sb = consts.tile([128, 8, D], BF16, tag="wk")
        wv_sb = consts.tile([128, 8, D], BF16, tag="wv")
        wo_sb = consts.tile([128, 8, D], BF16, tag="wo")
        bq_sb = consts.tile([128, 8], F32, tag="bq")
        bk_sb = consts.tile([128, 8], F32, tag="bk")
        bv_sb = consts.tile([128, H, DH], BF16, tag="bv")
        bo_sb = consts.tile([128, D], BF16, tag="bo")
        m3_sb = consts.tile([128, 384], BF16, tag="m3")
        i128_sb = consts.tile([128, 128], BF16, tag="i128")
        ones_sb = consts.tile([128, 64], BF16, tag="ones64")
        # persistent padded-K stationaries, double-buffered by chunk parity;
        # the zero halves are written once here and never touched again.
        ktp = []
        for i in range(2):
            ktp_i = consts.tile([128, H, CHUNK], BF16, tag=f"ktp{i}",
                                name=f"ktp{i}")
            ktp.append(ktp_i)
        for t, ap in ((wq_sb, wq_ap), (wk_sb, wk_ap), (wv_sb, wv_ap),
                      (wo_sb, wo_ap), (bq_sb, bq_ap), (bk_sb, bk_ap),
                      (bv_sb, bv_ap), (bo_sb, bo_ap), (m3_sb, m3_ap),
                      (i128_sb, i128_ap), (ones_sb, ones_ap)):
            nc.sync.dma_start(t[:], ap[:])
        for t in ktp:
            nc.vector.memset(t[:], 0.0)

        rep_iter = [None] if use_loop else list(range(repeat))
        loop_cm = tc.For_i(
            0, repeat, 1,
            hint_engines=(mybir.EngineType.PE, mybir.EngineType.DVE,
                          mybir.EngineType.Activation, mybir.EngineType.SP,
                          mybir.EngineType.Pool),
        ) if use_loop else None
        if loop_cm is not None:
            loop_cm.__enter__()
        for _rep in rep_iter:
            for c in range(N_CHUNK):
                with ExitStack() as chunk_ctx:
                    p_x = chunk_ctx.enter_context(
                        tc.tile_pool(name="p_x", bufs=2))
                    p_qt = chunk_ctx.enter_context(
                        tc.tile_pool(name="p_qt", bufs=2))
                    p_v3 = chunk_ctx.enter_context(
                        tc.tile_pool(name="p_v3", bufs=2))
                    p_pt = chunk_ctx.enter_context(
                        tc.tile_pool(name="p_pt", bufs=4))
                    p_rec = chunk_ctx.enter_context(
                        tc.tile_pool(name="p_rec", bufs=4))
                    p_y = chunk_ctx.enter_context(
                        tc.tile_pool(name="p_y", bufs=2))
                    ps_pj = chunk_ctx.enter_context(
                        tc.tile_pool(name="ps_pj", bufs=2, space="PSUM"))
                    ps_st = chunk_ctx.enter_context(
                        tc.tile_pool(name="ps_st", bufs=2, space="PSUM"))
                    ps_pl = chunk_ctx.enter_context(
                        tc.tile_pool(name="ps_pl", bufs=4, space="PSUM"))

                    kt_sb = ktp[c % 2]
                    xt_sb = p_x.tile([128, 8, CHUNK], BF16, tag="xa")
                    for kk in range(2):
                        nc.sync.dma_start(
                            xt_sb[:, 4 * kk:4 * (kk + 1), :],
                            xt_ap[c, :, 4 * kk:4 * (kk + 1), :])

                    qt_sb = p_qt.tile([128, 8, CHUNK], BF16, tag="qt")
                    v3_sb = p_v3.tile([128, NT, H, DH], BF16, tag="v3")

                    # ---- K / Q projections (feature-major) ----
                    if "qk" not in phases:
                        nc.vector.memset(qt_sb[:], 0.01)
                    for m in range(8 if "qk" in phases else 0):
                        acc = ps_pj.tile([128, CHUNK], F32, tag="acc")
                        for k in range(8):
                            nc.tensor.matmul(
                                acc[:], wk_sb[:, k, m * 128:(m + 1) * 128],
                                xt_sb[:, k, :],
                                start=(k == 0), stop=(k == 7))
                        nc.scalar.activation(
                            kt_sb[0:64, 2 * m, :], acc[0:64, :],
                            mybir.ActivationFunctionType.Identity,
                            bias=bk_sb[0:64, m:m + 1])
                        nc.scalar.activation(
                            kt_sb[64:128, 2 * m + 1, :], acc[64:128, :],
                            mybir.ActivationFunctionType.Identity,
                            bias=bk_sb[64:128, m:m + 1])
                        acc = ps_pj.tile([128, CHUNK], F32, tag="acc")
                        for k in range(8):
                            nc.tensor.matmul(
                                acc[:], wq_sb[:, k, m * 128:(m + 1) * 128],
                                xt_sb[:, k, :],
                                start=(k == 0), stop=(k == 7))
                        nc.scalar.activation(
                            qt_sb[:, m, :], acc[:],
                            mybir.ActivationFunctionType.Identity,
                            bias=bq_sb[:, m:m + 1])

                    # ---- V projection (token-major) ----
                    if "v" not in phases:
                        nc.vector.memset(v3_sb[:], 0.01)
                    for i in range(NT if "v" in phases else 0):
                        for h2 in range(2):
                            acc = ps_pj.tile([128, CHUNK], F32, tag="acc")
                            for k in range(8):
                                nc.tensor.matmul(
                                    acc[:],
                                    xt_sb[:, k, i * 128:(i + 1) * 128],
                                    wv_sb[:, k, h2 * 512:(h2 + 1) * 512],
                                    start=(k == 0), stop=(k == 7))
                            nc.vector.tensor_add(
                                v3_sb[:, i, h2 * 8:(h2 + 1) * 8, :],
                                acc[:].rearrange("p (h d) -> p h d", h=8),
                                bv_sb[:, h2 * 8:(h2 + 1) * 8, :])

                    # ---- attention ----
                    at_sb = p_x.tile([128, 8, CHUNK], BF16, tag="xa")
                    if "attn" not in phases:
                        nc.vector.memset(at_sb[:, 0, 0:4], 0.0)
                    for b in range(NB if "attn" in phases else 0):
                        t0 = b * W
                        for h in range(H):
                            hp = (h % 2) * 64
                            j = h // 2
                            # scores S^T packed [k,q]: cols 0:256 = k 0:128 x
                            # q 0:256; cols 256:384 = k 128:256 x q 128:256.
                            # causal -1e30 pattern seeded by matmul(I, m3).
                            st = ps_st.tile([128, 384], F32, tag="st")
                            nc.tensor.matmul(
                                st[:], i128_sb[:], m3_sb[:],
                                start=True, stop=False)
                            nc.tensor.matmul(
                                st[:, 0:W], kt_sb[:, h, t0:t0 + 128],
                                qt_sb[:, j, t0:t0 + W],
                                start=False, stop=False)
                            nc.tensor.matmul(
                                st[:, W:W + 128],
                                kt_sb[:, h, t0 + 128:t0 + W],
                                qt_sb[:, j, t0 + 128:t0 + W],
                                start=False, stop=True)
                            pt = p_pt.tile([128, 384], BF16, tag="pt")
                            nc.scalar.activation(
                                pt[:], st[:],
                                mybir.ActivationFunctionType.Exp,
                                scale=SCALE)
                            # pvls bank: cols 0:256 = P^T V, cols 256:512 =
                            # l broadcast to 64 partitions (ones stationary)
                            pvls = ps_pl.tile([64, 2 * W], F32, tag="pvls")
                            nc.tensor.matmul(
                                pvls[:, 0:W], v3_sb[:, 2 * b, h, :],
                                pt[:, 0:W], start=True, stop=False)
                            nc.tensor.matmul(
                                pvls[:, 128:W], v3_sb[:, 2 * b + 1, h, :],
                                pt[:, W:W + 128], start=False, stop=True)
                            nc.tensor.matmul(
                                pvls[:, W:2 * W], ones_sb[:], pt[:, 0:W],
                                start=True, stop=False)
                            nc.tensor.matmul(
                                pvls[:, W + 128:2 * W], ones_sb[:],
                                pt[:, W:W + 128], start=False, stop=True)
                            rec = p_rec.tile([64, W], F32, tag="rec")
                            nc.vector.reciprocal(rec[:], pvls[:, W:2 * W])
                            nc.vector.tensor_mul(
                                at_sb[hp:hp + 64, j, t0:t0 + W],
                                pvls[:, 0:W], rec[:])

                    # ---- output projection ----
                    if "o" not in phases:
                        continue
                    for i in range(NT):
                        y_t = p_y.tile([128, D], F32, tag="y")
                        for h2 in range(2):
                            acc = ps_pj.tile([128, CHUNK], F32, tag="acc")
                            for k in range(8):
                                nc.tensor.matmul(
                                    acc[:],
                                    at_sb[:, k, i * 128:(i + 1) * 128],
                                    wo_sb[:, k, h2 * 512:(h2 + 1) * 512],
                                    start=(k == 0), stop=(k == 7))
                            nc.vector.tensor_add(
                                y_t[:, h2 * 512:(h2 + 1) * 512], acc[:],
                                bo_sb[:, h2 * 512:(h2 + 1) * 512])
                        nc.sync.dma_start(
                            y_ap[c * CHUNK + i * 128:c * CHUNK + (i + 1) * 128, :],
                            y_t[:])
        if loop_cm is not None:
            loop_cm.__exit__(None, None, None)
    return nc


# ---------------------------------------------------------------------------
_CACHE: dict = {}


def _host_prep(x, Wq, bq, Wk, bk, Wv, bv, Wo, bo):
    x = np.asarray(x, np.float32)
    Wq, Wk, Wv, Wo = (np.asarray(w, np.float32) for w in (Wq, Wk, Wv, Wo))
    bq, bk, bv, bo = (np.asarray(b, np.float32) for b in (bq, bk, bv, bo))
    x_flat = np.ascontiguousarray(x.reshape(B * S, D))

    m3 = np.zeros((128, 384), np.float32)
    for p in range(128):
        m3[p, :p] = NEG
        m3[p, 256:256 + p] = NEG

    def wfmt(Wm):
        # [128, 8, D]: wfmt[p, k, c] = W[k*128 + p, c]
        return np.ascontiguousarray(
            Wm.reshape(8, 128, D).transpose(1, 0, 2)).astype(NPBF16)

    # xt[core, c, p, k, t] = x_flat[core*T_CORE + c*CHUNK + t, k*128 + p]
    xt = x_flat.T.reshape(8, 128, N_CORES, N_CHUNK, CHUNK)
    xt = np.ascontiguousarray(xt.transpose(2, 3, 1, 0, 4)).astype(NPBF16)

    shard = {"xt": xt}
    repl = {
        "wq": wfmt(Wq),
        "wk": wfmt(Wk),
        "wv": wfmt(Wv),
        "wo": wfmt(Wo),
        "bqr": np.ascontiguousarray(bq.reshape(8, 128).T),
        "bkr": np.ascontiguousarray(bk.reshape(8, 128).T),
        "bvh": np.ascontiguousarray(
            np.broadcast_to(bv.reshape(H, DH), (128, H, DH))).astype(NPBF16),
        "bob": np.ascontiguousarray(
            np.broadcast_to(bo, (128, D))).astype(NPBF16),
        "m3": m3.astype(NPBF16),
        "i128": np.eye(128, dtype=np.float32).astype(NPBF16),
        "ones64": np.ones((128, 64), np.float32).astype(NPBF16),
    }
    return shard, repl


def _make_runner(repeat: int, use_loop: bool = False,
                 phases=("qk", "v", "attn", "o")):
    """Build program + cached jitted executable. Returns (run, n_outs info)."""
    import jax
    from jax.sharding import Mesh, PartitionSpec
    from jax.experimental.shard_map import shard_map
    from concourse import bass2jax
    from concourse.bass2jax import _bass_exec_p, install_neuronx_cc_hook

    install_neuronx_cc_hook()
    nc = build_program(repeat, use_loop, phases)
    partition_name = (
        nc.partition_id_tensor.name if nc.partition_id_tensor else None
    )
    in_names, out_names, out_avals = [], [], []
    import jax.core
    for alloc in nc.m.functions[0].allocations:
        if not isinstance(alloc, mybir.MemoryLocationSet):
            continue
        name = alloc.memorylocations[0].name
        if alloc.kind == "ExternalInput":
            if name != partition_name:
                in_names.append(name)
        elif alloc.kind == "ExternalOutput":
            out_names.append(name)
            out_avals.append(jax.core.ShapedArray(
                tuple(alloc.tensor_shape), mybir.dt.np(alloc.dtype)))
    all_in_names = list(in_names) + list(out_names)
    if partition_name is not None:
        all_in_names.append(partition_name)

    def _body(*args):
        operands = list(args)
        if partition_name is not None:
            operands.append(bass2jax.partition_id_tensor())
        return tuple(_bass_exec_p.bind(
            *operands,
            out_avals=tuple(out_avals),
            in_names=tuple(all_in_names),
            out_names=tuple(out_names),
            lowering_input_output_aliases=(),
            sim_require_finite=True,
            sim_require_nnan=True,
            nc=nc,
        ))

    import jax as _jax
    devices = _jax.devices()[:N_CORES]
    mesh = Mesh(np.asarray(devices), ("core",))
    SHARDED_INS = {"xt"}
    in_specs = tuple(
        PartitionSpec("core") if n in SHARDED_INS else PartitionSpec()
        for n in in_names
    ) + (PartitionSpec("core"),) * len(out_names)
    out_specs = (PartitionSpec("core"),) * len(out_names)
    sharded = _jax.jit(
        shard_map(_body, mesh=mesh, in_specs=in_specs,
                  out_specs=out_specs, check_rep=False),
        keep_unused=True,
    )

    from jax.sharding import NamedSharding
    sh_core = NamedSharding(mesh, PartitionSpec("core"))
    sh_repl = NamedSharding(mesh, PartitionSpec())

    def _args(shard_arrs: dict, repl_arrs: dict):
        args, shs = [], []
        for n in in_names:
            if n in SHARDED_INS:
                a = shard_arrs[n]
                args.append(a.reshape(a.shape[0] * a.shape[1], *a.shape[2:]))
                shs.append(sh_core)
            else:
                args.append(repl_arrs[n])
                shs.append(sh_repl)
        for av in out_avals:
            args.append(np.zeros((N_CORES * av.shape[0], *av.shape[1:]),
                                 av.dtype))
            shs.append(sh_core)
        return args, shs

    class Runner:
        def stage(self, shard_arrs, repl_arrs):
            args, shs = _args(shard_arrs, repl_arrs)
            dargs = [_jax.device_put(a, s) for a, s in zip(args, shs)]
            _jax.block_until_ready(dargs)
            return dargs

        def exec_staged(self, dargs):
            outs = sharded(*dargs)
            _jax.block_until_ready(outs)
            return outs

        def run(self, shard_arrs, repl_arrs):
            args, _ = _args(shard_arrs, repl_arrs)
            outs = sharded(*args)
            _jax.block_until_ready(outs)
            return {
                name: np.asarray(outs[i]).reshape(N_CORES, *out_avals[i].shape)
                for i, name in enumerate(out_names)
            }

    return Runner()


def get_runner(repeat: int = 1, use_loop: bool = False,
               phases=("qk", "v", "attn", "o")):
    key = ("runner", repeat, use_loop, tuple(phases))
    if key not in _CACHE:
        _CACHE[key] = _make_runner(repeat, use_loop, phases)
    return _CACHE[key]


def kernel(**inputs) -> np.ndarray:
    runner = get_runner(repeat=1)
    shard, repl = _host_prep(**inputs)
    out = runner.run(shard, repl)
    y = out["y"].reshape(B * S, D)
    return y.reshape(B, S, D).astype(np.float32)
